# revision 5
# baseline (speedup 1.0000x reference)
"""Trainium2 Bass kernel for nn_IntraAttention.

reference math (per full problem):
    proj = h_t @ W.T                                    # [bs, dim]
    E    = einsum('bd,nbd->bn', proj, h)                # [bs, n]
    next_hist = concat([attn_history, E[None]], 0)      # [t+1, bs, n]
    M    = next_hist.max(0)                             # [bs, n]
    E2   = exp(E - M) / sum(exp(attn_history - M), 0)   # [bs, n]
    alpha = E2 / E2.sum(1, keepdims=True)               # [bs, n]
    C_t  = einsum('bn,nbd->bd', alpha, h)               # [bs, dim]
    returns (C_t, alpha, next_hist)

Strategy: batch-parallel over 8 NeuronCores (4 batch rows per core).
Per core, h is streamed once ([n,b,d] layout, n on partitions, 128-row
tiles).  E is computed with fused DVE scalar_tensor_tensor
(multiply by broadcast proj + free-axis reduce).  The temporal softmax
uses the algebraic identity
    sum_t exp(hist[t]-M) == exp(-M) * sum_t exp(hist[t])
with M' = clip(E, 8, 87) elementwise.  M only affects rounding (it
cancels exactly in real arithmetic); the clip bounds keep every
intermediate inside the normal f32 range so overflow/underflow behavior
(inf rows in E2, like the reference produces) matches IEEE f32 at the
same thresholds.  C_t and the alpha normalizer S are accumulated with
PE matmuls (weights = E2 column, streaming h), then the final divides
by S happen on the host in IEEE f32 (matching numpy/jax-cpu semantics
for the inf/NaN rows the reference generates).
"""

import numpy as np

BS, N, DIM, T_HIST = 32, 4096, 512, 16
NCORES = 8
BSH = BS // NCORES            # 4 batch rows per core
NT = N // 128                 # 32 n-tiles of 128
NGROUPS = 8                   # groups of 4 tiles
GT = NT // NGROUPS            # tiles per group
LO, HI = 8.0, 87.0            # clip bounds for M'
KSCALE = float(2.0 ** -64)    # exact pow2 weight scale: keeps C psum finite

_CACHE = {}


def _build():
    import concourse.tile as tile
    from concourse import bacc, mybir
    from contextlib import ExitStack

    DT = mybir.dt.float32
    A = mybir.AluOpType
    F = mybir.ActivationFunctionType

    nc = bacc.Bacc(None, target_bir_lowering=False, debug=False)

    h_d = nc.dram_tensor("h_in", [N, BSH, DIM], DT, kind="ExternalInput")
    hist_d = nc.dram_tensor("hist_in", [T_HIST, BSH, N], DT, kind="ExternalInput")
    ht_d = nc.dram_tensor("ht_in", [BSH, DIM], DT, kind="ExternalInput")
    w_d = nc.dram_tensor("w_in", [DIM, DIM], DT, kind="ExternalInput")
    sel_d = nc.dram_tensor("sel_in", [T_HIST * BSH, BSH], DT, kind="ExternalInput")
    ident_d = nc.dram_tensor("ident_in", [128, 128], DT, kind="ExternalInput")
    onesr_d = nc.dram_tensor("onesr_in", [1, 128], DT, kind="ExternalInput")

    e_out = nc.dram_tensor("e_out", [128, 128], DT, kind="ExternalOutput")
    e2_out = nc.dram_tensor("e2_out", [128, 128], DT, kind="ExternalOutput")
    cu_out = nc.dram_tensor("cu_out", [BSH, BSH * DIM], DT, kind="ExternalOutput")

    with tile.TileContext(nc) as tc, ExitStack() as ctx:
        singles = ctx.enter_context(tc.tile_pool(name="singles", bufs=1))

        ident = singles.tile([128, 128], DT)
        nc.sync.dma_start(out=ident, in_=ident_d[:, :])
        onesr = singles.tile([1, 128], DT)
        nc.sync.dma_start(out=onesr, in_=onesr_d[:, :])
        sel = singles.tile([T_HIST * BSH, BSH], DT)
        nc.sync.dma_start(out=sel, in_=sel_d[:, :])
        hist = singles.tile([T_HIST * BSH, N], DT)
        nc.sync.dma_start(out=hist, in_=hist_d.rearrange("t b n -> (t b) n"))
        htsb = singles.tile([BSH, DIM], DT)
        nc.sync.dma_start(out=htsb, in_=ht_d[:, :])

        p_bcast = singles.tile([128, BSH * DIM], DT)   # proj replicated on all partitions
        ut_all = singles.tile([128, 128], DT)          # U^T: [n-part, tile*4+b]
        e_all = singles.tile([128, 128], DT)           # E:   [n-part, tile*4+b]
        e2_all = singles.tile([128, 128], DT)          # E2
        e2k_all = singles.tile([128, 128], DT)         # E2 * 2**-64 (matmul weights)
        exph = singles.tile([T_HIST * BSH, N], DT)

        # ---------- prologue: proj = h_t @ W.T, broadcast; U^T ----------
        with (
            tc.tile_pool(name="prol", bufs=1) as prol,
            tc.tile_pool(name="prolps", bufs=2, space="PSUM") as prolps,
        ):
            wsb = prol.tile([128, 4, DIM], DT)  # [d%128, d//128, k]
            nc.sync.dma_start(out=wsb, in_=w_d.rearrange("(dc p) k -> p dc k", p=128))
            wt = prol.tile([128, 4, DIM], DT)   # [k%128, k//128, d]
            for dc in range(4):
                for kc in range(4):
                    pps = prolps.tile([128, 512], DT, tag="pps")
                    nc.tensor.transpose(
                        pps[:, 0:128], wsb[:, dc, kc * 128:(kc + 1) * 128], ident
                    )
                    nc.scalar.copy(wt[:, kc, dc * 128:(dc + 1) * 128], pps[:, 0:128])
            htt = prol.tile([128, 4, BSH], DT)  # [k%128, k//128, b]
            for kc in range(4):
                pps = prolps.tile([128, 512], DT, tag="pps")
                nc.tensor.transpose(
                    pps[:, 0:BSH], htsb[0:BSH, kc * 128:(kc + 1) * 128],
                    ident[0:BSH, 0:BSH],
                )
                nc.scalar.copy(htt[:, kc, :], pps[:, 0:BSH])
            projps = prolps.tile([BSH, DIM], DT, tag="projps", bufs=1)
            for kc in range(4):
                nc.tensor.matmul(
                    projps, lhsT=htt[:, kc, :], rhs=wt[:, kc, :],
                    start=(kc == 0), stop=(kc == 3), skip_group_check=True,
                )
            proj = prol.tile([BSH, DIM], DT)
            nc.scalar.copy(proj, projps)
            projflat = prol.tile([1, BSH * DIM], DT)
            for b in range(BSH):
                nc.sync.dma_start(
                    out=projflat[0:1, b * DIM:(b + 1) * DIM], in_=proj[b:b + 1, :]
                )
            for c in range(4):
                pps = prolps.tile([128, 512], DT, tag="pps")
                nc.tensor.matmul(
                    pps, lhsT=onesr, rhs=projflat[0:1, c * 512:(c + 1) * 512],
                    start=True, stop=True, skip_group_check=True,
                )
                nc.vector.tensor_copy(p_bcast[:, c * 512:(c + 1) * 512], pps)

            # U^T[n, b] = sum_t exp(hist[t, b, n]) via exp + selector matmul
            nc.scalar.activation(out=exph, in_=hist, func=F.Exp)
            utps = prolps.tile([128, 128], DT, tag="utps", bufs=1)
            for t in range(NT):
                nc.tensor.matmul(
                    utps[:, t * BSH:(t + 1) * BSH],
                    lhsT=exph[:, t * 128:(t + 1) * 128], rhs=sel,
                    start=True, stop=True, skip_group_check=True,
                )
            nc.vector.tensor_copy(ut_all, utps)

        # ---------- main loop over groups of 4 n-tiles ----------
        mainps = ctx.enter_context(tc.tile_pool(name="mainps", bufs=1, space="PSUM"))
        ps_c = [mainps.tile([BSH, DIM], DT, name=f"ps_c{b}") for b in range(BSH)]
        hpool = ctx.enter_context(tc.tile_pool(name="hpool", bufs=2))
        spool = ctx.enter_context(tc.tile_pool(name="spool", bufs=3))
        gpool = ctx.enter_context(tc.tile_pool(name="gpool", bufs=2))

        hv = h_d.rearrange("(g tt p) b d -> g p tt (b d)", tt=GT, p=128)

        for g in range(NGROUPS):
            hgrp = hpool.tile([128, GT, BSH * DIM], DT, tag="hgrp")
            nc.sync.dma_start(out=hgrp, in_=hv[g])
            # E for the 16 (tile, b) pairs of this group
            for tt in range(GT):
                t = g * GT + tt
                for b in range(BSH):
                    prod = spool.tile([128, DIM], DT, tag="prod")
                    nc.vector.scalar_tensor_tensor(
                        out=prod,
                        in0=hgrp[:, tt, b * DIM:(b + 1) * DIM],
                        scalar=1.0,
                        in1=p_bcast[:, b * DIM:(b + 1) * DIM],
                        op0=A.mult, op1=A.mult,
                        accum_out=e_all[:, t * BSH + b:t * BSH + b + 1],
                    )
            # softmax pieces, batched over the group's 16 columns
            sl = slice(g * GT * BSH, (g + 1) * GT * BSH)
            mg = gpool.tile([128, GT * BSH], DT, tag="mg")
            nc.vector.tensor_scalar(
                out=mg, in0=e_all[:, sl], scalar1=LO, scalar2=HI,
                op0=A.max, op1=A.min,
            )
            tnum = gpool.tile([128, GT * BSH], DT, tag="tnum")
            nc.vector.scalar_tensor_tensor(
                out=tnum, in0=e_all[:, sl], scalar=0.0, in1=mg,
                op0=A.add, op1=A.subtract,
            )
            numer = gpool.tile([128, GT * BSH], DT, tag="numer")
            nc.scalar.activation(out=numer, in_=tnum, func=F.Exp)
            zg = gpool.tile([128, GT * BSH], DT, tag="zg")
            nc.scalar.activation(out=zg, in_=mg, func=F.Exp, scale=-1.0)
            dg = gpool.tile([128, GT * BSH], DT, tag="dg")
            nc.vector.scalar_tensor_tensor(
                out=dg, in0=zg, scalar=1.0, in1=ut_all[:, sl],
                op0=A.mult, op1=A.mult,
            )
            rdg = gpool.tile([128, GT * BSH], DT, tag="rdg")
            nc.vector.reciprocal(out=rdg, in_=dg)
            nc.vector.scalar_tensor_tensor(
                out=e2_all[:, sl], in0=numer, scalar=1.0, in1=rdg,
                op0=A.mult, op1=A.mult,
            )
            nc.vector.tensor_scalar_mul(e2k_all[:, sl], e2_all[:, sl], KSCALE)
            # C_t / S accumulation
            for tt in range(GT):
                t = g * GT + tt
                lw = e2k_all[:, t * BSH:(t + 1) * BSH]
                for b in range(BSH):
                    nc.tensor.matmul(
                        ps_c[b], lhsT=lw, rhs=hgrp[:, tt, b * DIM:(b + 1) * DIM],
                        start=(t == 0), stop=(t == NT - 1), skip_group_check=True,
                    )

        # ---------- epilogue ----------
        csb = singles.tile([BSH, BSH * DIM], DT)
        for b in range(BSH):
            nc.scalar.copy(csb[:, b * DIM:(b + 1) * DIM], ps_c[b])
        nc.sync.dma_start(out=cu_out[:, :], in_=csb)
        nc.sync.dma_start(out=e_out[:, :], in_=e_all)
        nc.sync.dma_start(out=e2_out[:, :], in_=e2_all)

    nc.compile()
    return nc


def _get_nc():
    if "nc" not in _CACHE:
        _CACHE["nc"] = _build()
    return _CACHE["nc"]


def _ensure_ntff_hook():
    """This container's antenv lacks axon_hooks; synthesize it from the
    ctypes hook factory in trn_agent_boot so trace=True can capture NTFF."""
    import sys, types
    if "antenv.axon_hooks" in sys.modules:
        return
    try:
        from trn_agent_boot.trn_boot import _ntff_profile_via_ctypes
        hook = _ntff_profile_via_ctypes("/opt/axon/libaxon_pjrt.so")
    except Exception:
        hook = None
    m = types.ModuleType("antenv.axon_hooks")
    m.get_axon_ntff_profile_hook = lambda: hook
    m.set_axon_ntff_profile_hook = lambda h: None
    sys.modules["antenv.axon_hooks"] = m
    try:
        import antenv
        antenv.axon_hooks = m
    except Exception:
        pass


def _run(h_t, h, attn_history, W, trace=False):
    from concourse.bass_utils import run_bass_kernel_spmd

    if trace:
        _ensure_ntff_hook()

    nc = _get_nc()
    sel = np.tile(np.eye(BSH, dtype=np.float32), (T_HIST, 1))  # [64, 4], p=t*4+b
    ident = np.eye(128, dtype=np.float32)
    onesr = np.ones((1, 128), dtype=np.float32)
    in_maps = []
    for i in range(NCORES):
        b0 = i * BSH
        in_maps.append({
            "h_in": np.ascontiguousarray(h[:, b0:b0 + BSH, :]),
            "hist_in": np.ascontiguousarray(attn_history[:, b0:b0 + BSH, :]),
            "ht_in": np.ascontiguousarray(h_t[b0:b0 + BSH, :]),
            "w_in": np.ascontiguousarray(W),
            "sel_in": sel,
            "ident_in": ident,
            "onesr_in": onesr,
        })
    res = run_bass_kernel_spmd(
        nc, in_maps, core_ids=list(range(NCORES)), trace=trace
    )
    E = np.empty((BS, N), np.float32)
    E2 = np.empty((BS, N), np.float32)
    Cu = np.empty((BS, DIM), np.float32)
    for i in range(NCORES):
        r = res.results[i]
        b0 = i * BSH
        # e_out[p, t*4+b] -> E[b0+b, t*128+p]
        ea = np.asarray(r["e_out"]).reshape(128, NT, BSH)
        E[b0:b0 + BSH, :] = ea.transpose(2, 1, 0).reshape(BSH, N)
        e2a = np.asarray(r["e2_out"]).reshape(128, NT, BSH)
        E2[b0:b0 + BSH, :] = e2a.transpose(2, 1, 0).reshape(BSH, N)
        cua = np.asarray(r["cu_out"]).reshape(BSH, BSH, DIM)
        for b in range(BSH):
            Cu[b0 + b, :] = cua[b, b, :]
    return E, E2, Cu, res


def kernel(h_t, h, attn_history, W):
    h_t = np.asarray(h_t, np.float32)
    h = np.asarray(h, np.float32)
    attn_history = np.asarray(attn_history, np.float32)
    W = np.asarray(W, np.float32)
    E, E2, Cu, _ = _run(h_t, h, attn_history, W)
    # Normalize on the host in IEEE f32 (matches numpy/jax-cpu semantics for
    # the inf rows the reference produces: alpha -> {0, NaN@inf}, C_t -> NaN).
    with np.errstate(all="ignore"):
        S = E2.sum(axis=1, keepdims=True, dtype=np.float32)
        alpha = (E2 / S).astype(np.float32)
        C_t = (Cu / (np.float32(KSCALE) * S)).astype(np.float32)
    bad = ~np.isfinite(S.ravel())
    if bad.any():
        # reference: einsum over an alpha row containing NaN -> whole row NaN
        C_t[bad, :] = np.nan
    next_hist = np.concatenate([attn_history, E[None]], axis=0)
    return C_t, alpha, next_hist


# revision 7
# speedup vs baseline: 1.0646x; 1.0646x over previous
"""Trainium2 Bass kernel for nn_IntraAttention.

reference math (per full problem):
    proj = h_t @ W.T                                    # [bs, dim]
    E    = einsum('bd,nbd->bn', proj, h)                # [bs, n]
    next_hist = concat([attn_history, E[None]], 0)      # [t+1, bs, n]
    M    = next_hist.max(0)                             # [bs, n]
    E2   = exp(E - M) / sum(exp(attn_history - M), 0)   # [bs, n]
    alpha = E2 / E2.sum(1, keepdims=True)               # [bs, n]
    C_t  = einsum('bn,nbd->bd', alpha, h)               # [bs, dim]
    returns (C_t, alpha, next_hist)

Strategy: batch-parallel over 8 NeuronCores (4 batch rows per core).
Per core, h is streamed once ([n,b,d] layout, n on partitions, 128-row
tiles).  E is computed with fused DVE scalar_tensor_tensor
(multiply by broadcast proj + free-axis reduce).  The temporal softmax
uses the algebraic identity
    sum_t exp(hist[t]-M) == exp(-M) * sum_t exp(hist[t])
with M' = clip(E, 8, 87) elementwise.  M only affects rounding (it
cancels exactly in real arithmetic); the clip bounds keep every
intermediate inside the normal f32 range so overflow/underflow behavior
(inf rows in E2, like the reference produces) matches IEEE f32 at the
same thresholds.  C_t and the alpha normalizer S are accumulated with
PE matmuls (weights = E2 column, streaming h), then the final divides
by S happen on the host in IEEE f32 (matching numpy/jax-cpu semantics
for the inf/NaN rows the reference generates).
"""

import numpy as np

BS, N, DIM, T_HIST = 32, 4096, 512, 16
NCORES = 8
BSH = BS // NCORES            # 4 batch rows per core
NT = N // 128                 # 32 n-tiles of 128
NGROUPS = 8                   # groups of 4 tiles
GT = NT // NGROUPS            # tiles per group
LO, HI = 8.0, 87.0            # clip bounds for M'
KSCALE = float(2.0 ** -64)    # exact pow2 weight scale: keeps C psum finite

_CACHE = {}


def _build():
    import concourse.tile as tile
    from concourse import bacc, mybir
    from contextlib import ExitStack

    DT = mybir.dt.float32
    A = mybir.AluOpType
    F = mybir.ActivationFunctionType

    nc = bacc.Bacc(None, target_bir_lowering=False, debug=False)

    h_d = nc.dram_tensor("h_in", [N, BSH, DIM], DT, kind="ExternalInput")
    hist_d = nc.dram_tensor("hist_in", [T_HIST, BSH, N], DT, kind="ExternalInput")
    ht_d = nc.dram_tensor("ht_in", [BSH, DIM], DT, kind="ExternalInput")
    w_d = nc.dram_tensor("w_in", [DIM, DIM], DT, kind="ExternalInput")
    sel_d = nc.dram_tensor("sel_in", [T_HIST * BSH, BSH], DT, kind="ExternalInput")
    ident_d = nc.dram_tensor("ident_in", [128, 128], DT, kind="ExternalInput")
    onesr_d = nc.dram_tensor("onesr_in", [1, 128], DT, kind="ExternalInput")

    e_out = nc.dram_tensor("e_out", [128, 128], DT, kind="ExternalOutput")
    e2_out = nc.dram_tensor("e2_out", [128, 128], DT, kind="ExternalOutput")
    cu_out = nc.dram_tensor("cu_out", [BSH, BSH * DIM], DT, kind="ExternalOutput")

    with tile.TileContext(nc) as tc, ExitStack() as ctx:
        singles = ctx.enter_context(tc.tile_pool(name="singles", bufs=1))

        ident = singles.tile([128, 128], DT)
        nc.sync.dma_start(out=ident, in_=ident_d[:, :])
        onesr = singles.tile([1, 128], DT)
        nc.sync.dma_start(out=onesr, in_=onesr_d[:, :])
        sel = singles.tile([T_HIST * BSH, BSH], DT)
        nc.sync.dma_start(out=sel, in_=sel_d[:, :])
        hist = singles.tile([T_HIST * BSH, N], DT)
        nc.sync.dma_start(out=hist, in_=hist_d.rearrange("t b n -> (t b) n"))
        htsb = singles.tile([BSH, DIM], DT)
        nc.sync.dma_start(out=htsb, in_=ht_d[:, :])

        p_bcast = singles.tile([128, BSH * DIM], DT)   # proj replicated on all partitions
        ut_all = singles.tile([128, 128], DT)          # U^T: [n-part, tile*4+b]
        e_all = singles.tile([128, 128], DT)           # E:   [n-part, tile*4+b]
        e2_all = singles.tile([128, 128], DT)          # E2
        e2k_all = singles.tile([128, 128], DT)         # E2 * 2**-64 (matmul weights)
        exph = singles.tile([T_HIST * BSH, N], DT)

        # ---------- prologue: proj = h_t @ W.T, broadcast; U^T ----------
        with (
            tc.tile_pool(name="prol", bufs=1) as prol,
            tc.tile_pool(name="prolps", bufs=2, space="PSUM") as prolps,
        ):
            wsb = prol.tile([128, 4, DIM], DT)  # [d%128, d//128, k]
            nc.sync.dma_start(out=wsb, in_=w_d.rearrange("(dc p) k -> p dc k", p=128))
            wt = prol.tile([128, 4, DIM], DT)   # [k%128, k//128, d]
            for dc in range(4):
                for kc in range(4):
                    pps = prolps.tile([128, 512], DT, tag="pps")
                    nc.tensor.transpose(
                        pps[:, 0:128], wsb[:, dc, kc * 128:(kc + 1) * 128], ident
                    )
                    nc.scalar.copy(wt[:, kc, dc * 128:(dc + 1) * 128], pps[:, 0:128])
            htt = prol.tile([128, 4, BSH], DT)  # [k%128, k//128, b]
            for kc in range(4):
                pps = prolps.tile([128, 512], DT, tag="pps")
                nc.tensor.transpose(
                    pps[:, 0:BSH], htsb[0:BSH, kc * 128:(kc + 1) * 128],
                    ident[0:BSH, 0:BSH],
                )
                nc.scalar.copy(htt[:, kc, :], pps[:, 0:BSH])
            projps = prolps.tile([BSH, DIM], DT, tag="projps", bufs=1)
            for kc in range(4):
                nc.tensor.matmul(
                    projps, lhsT=htt[:, kc, :], rhs=wt[:, kc, :],
                    start=(kc == 0), stop=(kc == 3), skip_group_check=True,
                )
            proj = prol.tile([BSH, DIM], DT)
            nc.scalar.copy(proj, projps)
            projflat = prol.tile([1, BSH * DIM], DT)
            for b in range(BSH):
                nc.sync.dma_start(
                    out=projflat[0:1, b * DIM:(b + 1) * DIM], in_=proj[b:b + 1, :]
                )
            for c in range(4):
                pps = prolps.tile([128, 512], DT, tag="pps")
                nc.tensor.matmul(
                    pps, lhsT=onesr, rhs=projflat[0:1, c * 512:(c + 1) * 512],
                    start=True, stop=True, skip_group_check=True,
                )
                nc.vector.tensor_copy(p_bcast[:, c * 512:(c + 1) * 512], pps)

            # U^T[n, b] = sum_t exp(hist[t, b, n]) via exp + selector matmul
            nc.scalar.activation(out=exph, in_=hist, func=F.Exp)
            utps = prolps.tile([128, 128], DT, tag="utps", bufs=1)
            for t in range(NT):
                nc.tensor.matmul(
                    utps[:, t * BSH:(t + 1) * BSH],
                    lhsT=exph[:, t * 128:(t + 1) * 128], rhs=sel,
                    start=True, stop=True, skip_group_check=True,
                )
            nc.vector.tensor_copy(ut_all, utps)

        # ---------- main loop over groups of 4 n-tiles ----------
        mainps = ctx.enter_context(tc.tile_pool(name="mainps", bufs=1, space="PSUM"))
        ps_c = [mainps.tile([BSH, DIM], DT, name=f"ps_c{b}") for b in range(BSH)]
        hpool = ctx.enter_context(tc.tile_pool(name="hpool", bufs=8))
        spool = ctx.enter_context(tc.tile_pool(name="spool", bufs=3))
        gpool = ctx.enter_context(tc.tile_pool(name="gpool", bufs=2))

        hv = h_d.rearrange("(t p) b d -> t p (b d)", p=128)

        for g in range(NGROUPS):
            htiles = []
            for tt in range(GT):
                t = g * GT + tt
                htile = hpool.tile([128, BSH * DIM], DT, tag="htile")
                nc.sync.dma_start(out=htile, in_=hv[t])
                htiles.append(htile)
            # E for the 16 (tile, b) pairs of this group
            for tt in range(GT):
                t = g * GT + tt
                for b in range(BSH):
                    prod = spool.tile([128, DIM], DT, tag="prod")
                    nc.vector.scalar_tensor_tensor(
                        out=prod,
                        in0=htiles[tt][:, b * DIM:(b + 1) * DIM],
                        scalar=1.0,
                        in1=p_bcast[:, b * DIM:(b + 1) * DIM],
                        op0=A.mult, op1=A.mult,
                        accum_out=e_all[:, t * BSH + b:t * BSH + b + 1],
                    )
            # softmax pieces, batched over the group's 16 columns
            sl = slice(g * GT * BSH, (g + 1) * GT * BSH)
            mg = gpool.tile([128, GT * BSH], DT, tag="mg")
            nc.vector.tensor_scalar(
                out=mg, in0=e_all[:, sl], scalar1=LO, scalar2=HI,
                op0=A.max, op1=A.min,
            )
            tnum = gpool.tile([128, GT * BSH], DT, tag="tnum")
            nc.vector.scalar_tensor_tensor(
                out=tnum, in0=e_all[:, sl], scalar=0.0, in1=mg,
                op0=A.add, op1=A.subtract,
            )
            numer = gpool.tile([128, GT * BSH], DT, tag="numer")
            nc.scalar.activation(out=numer, in_=tnum, func=F.Exp)
            zg = gpool.tile([128, GT * BSH], DT, tag="zg")
            nc.scalar.activation(out=zg, in_=mg, func=F.Exp, scale=-1.0)
            dg = gpool.tile([128, GT * BSH], DT, tag="dg")
            nc.vector.scalar_tensor_tensor(
                out=dg, in0=zg, scalar=1.0, in1=ut_all[:, sl],
                op0=A.mult, op1=A.mult,
            )
            rdg = gpool.tile([128, GT * BSH], DT, tag="rdg")
            nc.vector.reciprocal(out=rdg, in_=dg)
            nc.vector.scalar_tensor_tensor(
                out=e2_all[:, sl], in0=numer, scalar=1.0, in1=rdg,
                op0=A.mult, op1=A.mult,
            )
            nc.vector.tensor_scalar_mul(e2k_all[:, sl], e2_all[:, sl], KSCALE)
            # C_t accumulation
            for tt in range(GT):
                t = g * GT + tt
                lw = e2k_all[:, t * BSH:(t + 1) * BSH]
                for b in range(BSH):
                    nc.tensor.matmul(
                        ps_c[b], lhsT=lw, rhs=htiles[tt][:, b * DIM:(b + 1) * DIM],
                        start=(t == 0), stop=(t == NT - 1), skip_group_check=True,
                    )

        # ---------- epilogue ----------
        csb = singles.tile([BSH, BSH * DIM], DT)
        for b in range(BSH):
            nc.scalar.copy(csb[:, b * DIM:(b + 1) * DIM], ps_c[b])
        nc.sync.dma_start(out=cu_out[:, :], in_=csb)
        nc.sync.dma_start(out=e_out[:, :], in_=e_all)
        nc.sync.dma_start(out=e2_out[:, :], in_=e2_all)

    nc.compile()
    return nc


def _get_nc():
    if "nc" not in _CACHE:
        _CACHE["nc"] = _build()
    return _CACHE["nc"]


def _ensure_ntff_hook():
    """This container's antenv lacks axon_hooks; synthesize it from the
    ctypes hook factory in trn_agent_boot so trace=True can capture NTFF."""
    import sys, types
    if "antenv.axon_hooks" in sys.modules:
        return
    try:
        from trn_agent_boot.trn_boot import _ntff_profile_via_ctypes
        hook = _ntff_profile_via_ctypes("/opt/axon/libaxon_pjrt.so")
    except Exception:
        hook = None
    m = types.ModuleType("antenv.axon_hooks")
    m.get_axon_ntff_profile_hook = lambda: hook
    m.set_axon_ntff_profile_hook = lambda h: None
    sys.modules["antenv.axon_hooks"] = m
    try:
        import antenv
        antenv.axon_hooks = m
    except Exception:
        pass


def _run(h_t, h, attn_history, W, trace=False):
    from concourse.bass_utils import run_bass_kernel_spmd

    if trace:
        _ensure_ntff_hook()

    nc = _get_nc()
    sel = np.tile(np.eye(BSH, dtype=np.float32), (T_HIST, 1))  # [64, 4], p=t*4+b
    ident = np.eye(128, dtype=np.float32)
    onesr = np.ones((1, 128), dtype=np.float32)
    in_maps = []
    for i in range(NCORES):
        b0 = i * BSH
        in_maps.append({
            "h_in": np.ascontiguousarray(h[:, b0:b0 + BSH, :]),
            "hist_in": np.ascontiguousarray(attn_history[:, b0:b0 + BSH, :]),
            "ht_in": np.ascontiguousarray(h_t[b0:b0 + BSH, :]),
            "w_in": np.ascontiguousarray(W),
            "sel_in": sel,
            "ident_in": ident,
            "onesr_in": onesr,
        })
    res = run_bass_kernel_spmd(
        nc, in_maps, core_ids=list(range(NCORES)), trace=trace
    )
    E = np.empty((BS, N), np.float32)
    E2 = np.empty((BS, N), np.float32)
    Cu = np.empty((BS, DIM), np.float32)
    for i in range(NCORES):
        r = res.results[i]
        b0 = i * BSH
        # e_out[p, t*4+b] -> E[b0+b, t*128+p]
        ea = np.asarray(r["e_out"]).reshape(128, NT, BSH)
        E[b0:b0 + BSH, :] = ea.transpose(2, 1, 0).reshape(BSH, N)
        e2a = np.asarray(r["e2_out"]).reshape(128, NT, BSH)
        E2[b0:b0 + BSH, :] = e2a.transpose(2, 1, 0).reshape(BSH, N)
        cua = np.asarray(r["cu_out"]).reshape(BSH, BSH, DIM)
        for b in range(BSH):
            Cu[b0 + b, :] = cua[b, b, :]
    return E, E2, Cu, res


def kernel(h_t, h, attn_history, W):
    h_t = np.asarray(h_t, np.float32)
    h = np.asarray(h, np.float32)
    attn_history = np.asarray(attn_history, np.float32)
    W = np.asarray(W, np.float32)
    E, E2, Cu, _ = _run(h_t, h, attn_history, W)
    # Normalize on the host in IEEE f32 (matches numpy/jax-cpu semantics for
    # the inf rows the reference produces: alpha -> {0, NaN@inf}, C_t -> NaN).
    with np.errstate(all="ignore"):
        S = E2.sum(axis=1, keepdims=True, dtype=np.float32)
        alpha = (E2 / S).astype(np.float32)
        C_t = (Cu / (np.float32(KSCALE) * S)).astype(np.float32)
    bad = ~np.isfinite(S.ravel())
    if bad.any():
        # reference: einsum over an alpha row containing NaN -> whole row NaN
        C_t[bad, :] = np.nan
    next_hist = np.concatenate([attn_history, E[None]], axis=0)
    return C_t, alpha, next_hist


# revision 9
# speedup vs baseline: 1.4458x; 1.3581x over previous
"""Trainium2 Bass kernel for nn_IntraAttention.

reference math (per full problem):
    proj = h_t @ W.T                                    # [bs, dim]
    E    = einsum('bd,nbd->bn', proj, h)                # [bs, n]
    next_hist = concat([attn_history, E[None]], 0)      # [t+1, bs, n]
    M    = next_hist.max(0)                             # [bs, n]
    E2   = exp(E - M) / sum(exp(attn_history - M), 0)   # [bs, n]
    alpha = E2 / E2.sum(1, keepdims=True)               # [bs, n]
    C_t  = einsum('bn,nbd->bd', alpha, h)               # [bs, dim]
    returns (C_t, alpha, next_hist)

Strategy: batch-parallel over 8 NeuronCores (4 batch rows per core).
Per core, h is streamed once ([n,b,d] layout, n on partitions, 128-row
tiles).  E is computed with fused DVE scalar_tensor_tensor
(multiply by broadcast proj + free-axis reduce).  The temporal softmax
uses the algebraic identity
    sum_t exp(hist[t]-M) == exp(-M) * sum_t exp(hist[t])
with M' = clip(E, 8, 87) elementwise.  M only affects rounding (it
cancels exactly in real arithmetic); the clip bounds keep every
intermediate inside the normal f32 range so overflow/underflow behavior
(inf rows in E2, like the reference produces) matches IEEE f32 at the
same thresholds.  C_t and the alpha normalizer S are accumulated with
PE matmuls (weights = E2 column, streaming h), then the final divides
by S happen on the host in IEEE f32 (matching numpy/jax-cpu semantics
for the inf/NaN rows the reference generates).
"""

import numpy as np

BS, N, DIM, T_HIST = 32, 4096, 512, 16
NCORES = 8
BSH = BS // NCORES            # 4 batch rows per core
NT = N // 128                 # 32 n-tiles of 128
NGROUPS = 8                   # groups of 4 tiles
GT = NT // NGROUPS            # tiles per group
LO, HI = 8.0, 87.0            # clip bounds for M'
KSCALE = float(2.0 ** -64)    # exact pow2 weight scale: keeps C psum finite

_CACHE = {}


def _build():
    import concourse.tile as tile
    from concourse import bacc, mybir
    from contextlib import ExitStack

    DT = mybir.dt.float32
    A = mybir.AluOpType
    F = mybir.ActivationFunctionType

    nc = bacc.Bacc(None, target_bir_lowering=False, debug=False)

    h_d = nc.dram_tensor("h_in", [N, BSH, DIM], DT, kind="ExternalInput")
    hist_d = nc.dram_tensor("hist_in", [T_HIST, BSH, N], DT, kind="ExternalInput")
    ht_d = nc.dram_tensor("ht_in", [BSH, DIM], DT, kind="ExternalInput")
    w_d = nc.dram_tensor("w_in", [DIM, DIM], DT, kind="ExternalInput")
    sel_d = nc.dram_tensor("sel_in", [T_HIST * BSH, BSH], DT, kind="ExternalInput")
    ident_d = nc.dram_tensor("ident_in", [128, 128], DT, kind="ExternalInput")
    onesr_d = nc.dram_tensor("onesr_in", [1, 128], DT, kind="ExternalInput")

    e_out = nc.dram_tensor("e_out", [128, 128], DT, kind="ExternalOutput")
    e2_out = nc.dram_tensor("e2_out", [128, 128], DT, kind="ExternalOutput")
    cu_out = nc.dram_tensor("cu_out", [BSH, BSH * DIM], DT, kind="ExternalOutput")

    with tile.TileContext(nc) as tc, ExitStack() as ctx:
        singles = ctx.enter_context(tc.tile_pool(name="singles", bufs=1))

        ident = singles.tile([128, 128], DT)
        nc.sync.dma_start(out=ident, in_=ident_d[:, :])
        onesr = singles.tile([1, 128], DT)
        nc.sync.dma_start(out=onesr, in_=onesr_d[:, :])
        sel = singles.tile([T_HIST * BSH, BSH], DT)
        nc.sync.dma_start(out=sel, in_=sel_d[:, :])
        hist = singles.tile([T_HIST * BSH, N], DT)
        nc.sync.dma_start(out=hist, in_=hist_d.rearrange("t b n -> (t b) n"))
        htsb = singles.tile([BSH, DIM], DT)
        nc.sync.dma_start(out=htsb, in_=ht_d[:, :])

        p_bcast = singles.tile([128, BSH * DIM], DT)   # proj replicated on all partitions
        ut_all = singles.tile([128, 128], DT)          # U^T: [n-part, tile*4+b]
        e_all = singles.tile([128, 128], DT)           # E:   [n-part, tile*4+b]
        e2_all = singles.tile([128, 128], DT)          # E2
        e2k_all = singles.tile([128, 128], DT)         # E2 * 2**-64 (matmul weights)
        exph = singles.tile([T_HIST * BSH, N], DT)

        # ---------- prologue: proj = h_t @ W.T, broadcast; U^T ----------
        with (
            tc.tile_pool(name="prol", bufs=1) as prol,
            tc.tile_pool(name="prolps", bufs=2, space="PSUM") as prolps,
        ):
            wsb = prol.tile([128, 4, DIM], DT)  # [d%128, d//128, k]
            nc.sync.dma_start(out=wsb, in_=w_d.rearrange("(dc p) k -> p dc k", p=128))
            wt = prol.tile([128, 4, DIM], DT)   # [k%128, k//128, d]
            for dc in range(4):
                for kc in range(4):
                    pps = prolps.tile([128, 512], DT, tag="pps")
                    nc.tensor.transpose(
                        pps[:, 0:128], wsb[:, dc, kc * 128:(kc + 1) * 128], ident
                    )
                    nc.scalar.copy(wt[:, kc, dc * 128:(dc + 1) * 128], pps[:, 0:128])
            htt = prol.tile([128, 4, BSH], DT)  # [k%128, k//128, b]
            for kc in range(4):
                pps = prolps.tile([128, 512], DT, tag="pps")
                nc.tensor.transpose(
                    pps[:, 0:BSH], htsb[0:BSH, kc * 128:(kc + 1) * 128],
                    ident[0:BSH, 0:BSH],
                )
                nc.scalar.copy(htt[:, kc, :], pps[:, 0:BSH])
            projps = prolps.tile([BSH, DIM], DT, tag="projps", bufs=1)
            for kc in range(4):
                nc.tensor.matmul(
                    projps, lhsT=htt[:, kc, :], rhs=wt[:, kc, :],
                    start=(kc == 0), stop=(kc == 3), skip_group_check=True,
                )
            proj = prol.tile([BSH, DIM], DT)
            nc.scalar.copy(proj, projps)
            projflat = prol.tile([1, BSH * DIM], DT)
            for b in range(BSH):
                nc.sync.dma_start(
                    out=projflat[0:1, b * DIM:(b + 1) * DIM], in_=proj[b:b + 1, :]
                )
            for c in range(4):
                pps = prolps.tile([128, 512], DT, tag="pps")
                nc.tensor.matmul(
                    pps, lhsT=onesr, rhs=projflat[0:1, c * 512:(c + 1) * 512],
                    start=True, stop=True, skip_group_check=True,
                )
                nc.vector.tensor_copy(p_bcast[:, c * 512:(c + 1) * 512], pps)

            # U^T[n, b] = sum_t exp(hist[t, b, n]) via exp + selector matmul
            nc.scalar.activation(out=exph, in_=hist, func=F.Exp)
            utps = prolps.tile([128, 128], DT, tag="utps", bufs=1)
            for t in range(NT):
                nc.tensor.matmul(
                    utps[:, t * BSH:(t + 1) * BSH],
                    lhsT=exph[:, t * 128:(t + 1) * 128], rhs=sel,
                    start=True, stop=True, skip_group_check=True,
                )
            nc.vector.tensor_copy(ut_all, utps)

        # ---------- main loop over groups of 4 n-tiles ----------
        mainps = ctx.enter_context(tc.tile_pool(name="mainps", bufs=1, space="PSUM"))
        ps_call = mainps.tile([128, DIM], DT)  # row block 32b holds batch b's C
        hpool = ctx.enter_context(tc.tile_pool(name="hpool", bufs=8))
        spool = ctx.enter_context(tc.tile_pool(name="spool", bufs=3))
        gpool = ctx.enter_context(tc.tile_pool(name="gpool", bufs=2))

        hv = h_d.rearrange("(t p) b d -> t p (b d)", p=128)

        for g in range(NGROUPS):
            htiles = []
            for tt in range(GT):
                t = g * GT + tt
                htile = hpool.tile([128, BSH * DIM], DT, tag="htile")
                nc.sync.dma_start(out=htile, in_=hv[t])
                htiles.append(htile)
            # E for the 16 (tile, b) pairs of this group
            for tt in range(GT):
                t = g * GT + tt
                for b in range(BSH):
                    prod = spool.tile([128, DIM], DT, tag="prod")
                    nc.vector.scalar_tensor_tensor(
                        out=prod,
                        in0=htiles[tt][:, b * DIM:(b + 1) * DIM],
                        scalar=1.0,
                        in1=p_bcast[:, b * DIM:(b + 1) * DIM],
                        op0=A.mult, op1=A.mult,
                        accum_out=e_all[:, t * BSH + b:t * BSH + b + 1],
                    )
            # softmax pieces, batched over the group's 16 columns
            sl = slice(g * GT * BSH, (g + 1) * GT * BSH)
            mg = gpool.tile([128, GT * BSH], DT, tag="mg")
            nc.vector.tensor_scalar(
                out=mg, in0=e_all[:, sl], scalar1=LO, scalar2=HI,
                op0=A.max, op1=A.min,
            )
            tnum = gpool.tile([128, GT * BSH], DT, tag="tnum")
            nc.vector.scalar_tensor_tensor(
                out=tnum, in0=e_all[:, sl], scalar=0.0, in1=mg,
                op0=A.add, op1=A.subtract,
            )
            numer = gpool.tile([128, GT * BSH], DT, tag="numer")
            nc.scalar.activation(out=numer, in_=tnum, func=F.Exp)
            zg = gpool.tile([128, GT * BSH], DT, tag="zg")
            nc.scalar.activation(out=zg, in_=mg, func=F.Exp, scale=-1.0)
            dg = gpool.tile([128, GT * BSH], DT, tag="dg")
            nc.vector.scalar_tensor_tensor(
                out=dg, in0=zg, scalar=1.0, in1=ut_all[:, sl],
                op0=A.mult, op1=A.mult,
            )
            rdg = gpool.tile([128, GT * BSH], DT, tag="rdg")
            nc.vector.reciprocal(out=rdg, in_=dg)
            nc.vector.scalar_tensor_tensor(
                out=e2_all[:, sl], in0=numer, scalar=1.0, in1=rdg,
                op0=A.mult, op1=A.mult,
            )
            nc.vector.tensor_scalar_mul(e2k_all[:, sl], e2_all[:, sl], KSCALE)
            # C_t accumulation
            for tt in range(GT):
                t = g * GT + tt
                lw = e2k_all[:, t * BSH:(t + 1) * BSH]
                for b in range(BSH):
                    nc.tensor.matmul(
                        ps_call[32 * b:32 * b + BSH, :],
                        lhsT=lw, rhs=htiles[tt][:, b * DIM:(b + 1) * DIM],
                        start=(t == 0), stop=(t == NT - 1), skip_group_check=True,
                        tile_position=(0, 32 * b),
                    )

        # ---------- epilogue ----------
        csb = singles.tile([BSH, BSH * DIM], DT)
        for b in range(BSH):
            nc.scalar.copy(csb[:, b * DIM:(b + 1) * DIM], ps_call[32 * b:32 * b + BSH, :])
        nc.sync.dma_start(out=cu_out[:, :], in_=csb)
        nc.sync.dma_start(out=e_out[:, :], in_=e_all)
        nc.sync.dma_start(out=e2_out[:, :], in_=e2_all)

    nc.compile()
    return nc


def _get_nc():
    if "nc" not in _CACHE:
        _CACHE["nc"] = _build()
    return _CACHE["nc"]


def _ensure_ntff_hook():
    """This container's antenv lacks axon_hooks; synthesize it from the
    ctypes hook factory in trn_agent_boot so trace=True can capture NTFF."""
    import sys, types
    if "antenv.axon_hooks" in sys.modules:
        return
    try:
        from trn_agent_boot.trn_boot import _ntff_profile_via_ctypes
        hook = _ntff_profile_via_ctypes("/opt/axon/libaxon_pjrt.so")
    except Exception:
        hook = None
    m = types.ModuleType("antenv.axon_hooks")
    m.get_axon_ntff_profile_hook = lambda: hook
    m.set_axon_ntff_profile_hook = lambda h: None
    sys.modules["antenv.axon_hooks"] = m
    try:
        import antenv
        antenv.axon_hooks = m
    except Exception:
        pass


def _run(h_t, h, attn_history, W, trace=False):
    from concourse.bass_utils import run_bass_kernel_spmd

    if trace:
        _ensure_ntff_hook()

    nc = _get_nc()
    sel = np.tile(np.eye(BSH, dtype=np.float32), (T_HIST, 1))  # [64, 4], p=t*4+b
    ident = np.eye(128, dtype=np.float32)
    onesr = np.ones((1, 128), dtype=np.float32)
    in_maps = []
    for i in range(NCORES):
        b0 = i * BSH
        in_maps.append({
            "h_in": np.ascontiguousarray(h[:, b0:b0 + BSH, :]),
            "hist_in": np.ascontiguousarray(attn_history[:, b0:b0 + BSH, :]),
            "ht_in": np.ascontiguousarray(h_t[b0:b0 + BSH, :]),
            "w_in": np.ascontiguousarray(W),
            "sel_in": sel,
            "ident_in": ident,
            "onesr_in": onesr,
        })
    res = run_bass_kernel_spmd(
        nc, in_maps, core_ids=list(range(NCORES)), trace=trace
    )
    E = np.empty((BS, N), np.float32)
    E2 = np.empty((BS, N), np.float32)
    Cu = np.empty((BS, DIM), np.float32)
    for i in range(NCORES):
        r = res.results[i]
        b0 = i * BSH
        # e_out[p, t*4+b] -> E[b0+b, t*128+p]
        ea = np.asarray(r["e_out"]).reshape(128, NT, BSH)
        E[b0:b0 + BSH, :] = ea.transpose(2, 1, 0).reshape(BSH, N)
        e2a = np.asarray(r["e2_out"]).reshape(128, NT, BSH)
        E2[b0:b0 + BSH, :] = e2a.transpose(2, 1, 0).reshape(BSH, N)
        cua = np.asarray(r["cu_out"]).reshape(BSH, BSH, DIM)
        for b in range(BSH):
            Cu[b0 + b, :] = cua[b, b, :]
    return E, E2, Cu, res


def kernel(h_t, h, attn_history, W):
    h_t = np.asarray(h_t, np.float32)
    h = np.asarray(h, np.float32)
    attn_history = np.asarray(attn_history, np.float32)
    W = np.asarray(W, np.float32)
    E, E2, Cu, _ = _run(h_t, h, attn_history, W)
    # Normalize on the host in IEEE f32 (matches numpy/jax-cpu semantics for
    # the inf rows the reference produces: alpha -> {0, NaN@inf}, C_t -> NaN).
    with np.errstate(all="ignore"):
        S = E2.sum(axis=1, keepdims=True, dtype=np.float32)
        alpha = (E2 / S).astype(np.float32)
        C_t = (Cu / (np.float32(KSCALE) * S)).astype(np.float32)
    bad = ~np.isfinite(S.ravel())
    if bad.any():
        # reference: einsum over an alpha row containing NaN -> whole row NaN
        C_t[bad, :] = np.nan
    next_hist = np.concatenate([attn_history, E[None]], axis=0)
    return C_t, alpha, next_hist
